# revision 1
# baseline (speedup 1.0000x reference)
"""DDSP harmonic oscillator — Trainium2 Bass kernel (8-core data parallel).

Full inputs:
  f0_hz                (32, 64000) f32
  harmonic_amplitudes  (32, 64000, 65) f32
  phase                (32,) f32
Output: (32, 64000) f32

Sharding: batch rows 4 per core across 8 cores. Per core, per row:
  phase 1 (omega): t = (2pi*f0)/SR elementwise; high-precision blocked
    cumsum: (128 partitions x 500 samples), sequential fp32 scan per
    partition (tensor_tensor_scan), cross-partition carry via an exact
    split triangular matmul, compensated recombine -> omega = RN(true
    cumsum); r0 = omega mod 2pi (exact-product Cody-Waite); rc = r0/2pi.
  phase 2 (grid, chunks of T samples/partition, grouped by G for
    activation-table locality):
      ACT:   softplus(-x) -> exp(-log10*sp + ln2)  [= 2*sigmoid^log10],
             magic-round biases, Sin(2pi * r1c)
      DVE:   mask-mult, normalize sums, r1c = y - round(y), products
      GPSIMD: k*f0 and k*rc broadcast grids
"""

import math

import numpy as np

_CACHE = {}
_ACT_CHAIN = [True]

ROWS = 4          # batch rows per core
S = 64000         # samples
P = 128           # partitions
J = S // P        # 500 samples per partition
T = 25            # samples-per-partition per grid chunk
NCHUNK = J // T
G = 5             # chunks per activation-table group
CHAIN = "group"   # chain ACT ops within each group for table locality
HA_BUFS = 2
GRID_BUFS = 2
BF16 = False
POOL_MAGIC_N = 4   # all magic-rounds on gpsimd
H = 64            # harmonics
SR = 44100.0
LOG10 = math.log(10.0)
TWO_PI = 2.0 * math.pi

_f32 = np.float32


def _split_const(x, bits):
    f = _f32(x)
    _, e = math.frexp(float(f))
    q = 2.0 ** (e - bits)
    return float(_f32(round(float(f) / q) * q))


# stage-A Cody-Waite (m <= 409, 9 bits -> 13-bit leading constants)
A1 = _split_const(TWO_PI, 13)
A2 = _split_const(TWO_PI - A1, 13)
A3 = float(_f32(TWO_PI - A1 - A2))
INV2PI = float(_f32(1.0 / TWO_PI))
MAGIC = float(_f32(1.5 * 2.0 ** 23))
C2PI = float(_f32(TWO_PI))
RINV = float(_f32(1.0 / SR))
RINV_LO = float(_f32(1.0 / SR - np.float64(_f32(1.0 / SR))))
S15 = float(_f32(2.0 ** 15))
LN2 = float(_f32(math.log(2.0)))


def _build():
    import concourse.bacc as bacc
    import concourse.mybir as mybir
    import concourse.tile as tile
    from concourse.tile_rust import add_dep_helper

    dt = mybir.dt
    F32 = dt.float32
    AF = mybir.ActivationFunctionType
    ALU = mybir.AluOpType

    nc = bacc.Bacc("TRN2", target_bir_lowering=False, debug=False, num_devices=8)
    _prev_act = [None]

    def _act(*args, chain=True, **kwargs):
        ins = nc.scalar.activation(*args, **kwargs)
        if CHAIN == "none":
            chain = False
        if _prev_act[0] is not None and chain:
            add_dep_helper(ins.ins, _prev_act[0].ins, sync=False, reason="act order")
        if chain:
            _prev_act[0] = ins
        return ins
    f0_d = nc.dram_tensor("f0", [ROWS, S], F32, kind="ExternalInput")
    ha_d = nc.dram_tensor("ha", [ROWS, S, H + 1], F32, kind="ExternalInput")
    ph_d = nc.dram_tensor("ph", [ROWS, 1], F32, kind="ExternalInput")
    y_d = nc.dram_tensor("y", [ROWS, S], F32, kind="ExternalOutput")

    with tile.TileContext(nc) as tc:
        with tc.tile_pool(name="const", bufs=1) as cpool, \
             tc.tile_pool(name="row", bufs=2) as rpool, \
             tc.tile_pool(name="ha", bufs=HA_BUFS) as hpool, \
             tc.tile_pool(name="grid", bufs=GRID_BUFS) as gpool, \
             tc.tile_pool(name="psum", bufs=2, space="PSUM") as ppool:

            # ---- constants ----
            bneg = cpool.tile([P, 1], F32)
            nc.vector.memset(bneg[:], -MAGIC)
            bpos = cpool.tile([P, 1], F32)
            nc.vector.memset(bpos[:], MAGIC)
            bln2 = cpool.tile([P, 1], F32)
            nc.vector.memset(bln2[:], LN2)
            # harmonic grid 1..64 repeated T times
            harm_i = cpool.tile([P, T, H], dt.int32)
            nc.gpsimd.iota(harm_i[:], [[0, T], [1, H]], base=1, channel_multiplier=0)
            harm = cpool.tile([P, T, H], F32)
            nc.vector.tensor_copy(harm[:], harm_i[:])
            # strictly-lower-triangular ones (lhsT[i, p] = 1 if i < p)
            it_j = cpool.tile([P, P], dt.int32)
            nc.gpsimd.iota(it_j[:], [[1, P]], channel_multiplier=0)
            it_p = cpool.tile([P, 1], dt.int32)
            nc.gpsimd.iota(it_p[:], [[0, 1]], channel_multiplier=1)
            jf = cpool.tile([P, P], F32)
            nc.vector.tensor_copy(jf[:], it_j[:])
            pf = cpool.tile([P, 1], F32)
            nc.vector.tensor_copy(pf[:], it_p[:])
            tri = cpool.tile([P, P], F32)
            nc.vector.tensor_scalar(tri[:], jf[:], pf[:], None, ALU.is_gt)

            for r in range(ROWS):
                # ---- phase 1: omega ----
                F = rpool.tile([P, J], F32, tag="F")
                nc.sync.dma_start(F[:], f0_d[r].rearrange("(p j) -> p j", p=P))
                phs = rpool.tile([P, 1], F32, tag="phs")
                nc.sync.dma_start(phs[:], ph_d[r : r + 1, :].partition_broadcast(P))

                u = rpool.tile([P, J], F32, tag="u")
                nc.vector.tensor_scalar_mul(u[:], F[:], C2PI)
                q0 = rpool.tile([P, J], F32, tag="q0")
                nc.vector.tensor_scalar_mul(q0[:], u[:], RINV)
                t_row = q0
                nc.vector.scalar_tensor_tensor(
                    t_row[:], u[:], RINV_LO, q0[:], ALU.mult, ALU.add
                )
                L = rpool.tile([P, J], F32, tag="L")
                nc.vector.tensor_tensor_scan(
                    L[:], t_row[:], t_row[:], 0.0, ALU.add, ALU.bypass
                )
                # chunk totals, split, exclusive carry matmul
                rhs = rpool.tile([P, 2], F32, tag="rhs")
                nc.vector.tensor_scalar(
                    rhs[:, 0:1], L[:, J - 1 : J], S15, -S15, ALU.add, ALU.add
                )
                nc.vector.tensor_tensor(
                    rhs[:, 1:2], L[:, J - 1 : J], rhs[:, 0:1], ALU.subtract
                )
                pc = ppool.tile([P, 2], F32, tag="pc")
                nc.tensor.matmul(pc[:], tri[:], rhs[:])
                cc = rpool.tile([P, 2], F32, tag="cc")
                nc.vector.tensor_copy(cc[:], pc[:])
                # Fast2Sum(Chi, Clo) -> Ch, Cl
                Ch = rpool.tile([P, 1], F32, tag="Ch")
                nc.vector.tensor_tensor(Ch[:], cc[:, 0:1], cc[:, 1:2], ALU.add)
                z = rpool.tile([P, 1], F32, tag="z")
                nc.vector.tensor_tensor(z[:], Ch[:], cc[:, 0:1], ALU.subtract)
                Cl = rpool.tile([P, 1], F32, tag="Cl")
                nc.vector.tensor_tensor(Cl[:], cc[:, 1:2], z[:], ALU.subtract)
                # omega = RN(Ch + L + Cl), compensated
                s1 = rpool.tile([P, J], F32, tag="s1")
                nc.vector.tensor_scalar(s1[:], L[:], Ch[:], None, ALU.add)
                dtmp = rpool.tile([P, J], F32, tag="dtmp")
                nc.vector.tensor_scalar(dtmp[:], s1[:], Ch[:], None, ALU.subtract)
                e1 = dtmp
                nc.vector.tensor_tensor(e1[:], L[:], dtmp[:], ALU.subtract)
                e2 = e1
                nc.vector.tensor_scalar(e2[:], e1[:], Cl[:], None, ALU.add)
                om = s1
                nc.vector.tensor_tensor(om[:], s1[:], e2[:], ALU.add)
                a_row = om
                nc.vector.tensor_scalar(a_row[:], om[:], phs[:], None, ALU.add)
                # stage-A reduction: r0 = a mod 2pi, then rc = r0 / 2pi
                mA = rpool.tile([P, J], F32, tag="mA")
                nc.vector.tensor_scalar(
                    mA[:], a_row[:], INV2PI, MAGIC, ALU.mult, ALU.add
                )
                _act(mA[:], mA[:], AF.Identity, bias=bneg[:])
                r0 = L
                nc.vector.cody_waite_cascade(r0[:], a_row[:], mA[:], A1, A2, A3)
                rc = mA
                nc.vector.tensor_scalar_mul(rc[:], r0[:], INV2PI)

                SIG = rpool.tile([P, J], F32, tag="SIG")
                SS_row = rpool.tile([P, 2, J], F32, tag="SS_row")
                tot_row = rpool.tile([P, J], F32, tag="tot_row")

                # ---- phase 2: grid chunks, grouped for table locality ----
                for g0 in range(0, NCHUNK, G):
                    group = range(g0, min(g0 + G, NCHUNK))
                    if CHAIN.startswith("group"):
                        _prev_act[0] = None
                    HAs = {}
                    for c in group:
                        s0 = c * T
                        Hc = hpool.tile([P, T, H + 1], F32, tag=f"Hc{c - g0}")
                        nc.sync.dma_start(
                            Hc[:],
                            ha_d[r, :, :].rearrange("(p j) c -> p j c", p=P)[
                                :, s0 : s0 + T, :
                            ],
                        )
                        HAs[c] = Hc
                    # sigmoid for the whole group (one table set)
                    for c in group:
                        _act(HAs[c][:], HAs[c][:], AF.Sigmoid)
                    # 2*sigmoid^log10 = exp(log10*ln(sg) + ln2) (ln+exp share a set)
                    for c in group:
                        _act(HAs[c][:], HAs[c][:], AF.Ln)
                    for c in group:
                        _act(
                            HAs[c][:], HAs[c][:], AF.Exp, scale=LOG10, bias=bln2[:]
                        )
                    # remaining per-chunk work (Identity/Sin share one set)
                    for c in group:
                        s0 = c * T
                        HA = HAs[c]
                        h_v = HA[:][:, :, 1 : H + 1]
                        tot_v = HA[:][:, :, 0:1].rearrange("p t o -> p (t o)")
                        nc.gpsimd.tensor_copy(tot_row[:, s0 : s0 + T], tot_v)

                        X = gpool.tile([P, 2, T, H], F32, tag="X")
                        f0b = F[:][:, s0 : s0 + T].broadcast_to((P, T, H))
                        fk = gpool.tile([P, T, H], F32, tag="fk")
                        nc.gpsimd.tensor_tensor(fk[:], harm[:], f0b, ALU.mult)
                        ha_m = X[:][:, 0]
                        nc.vector.scalar_tensor_tensor(
                            ha_m, fk[:], SR / 2.0, h_v, ALU.is_lt, ALU.mult
                        )

                        rcb = rc[:][:, s0 : s0 + T].broadcast_to((P, T, H))
                        yg = gpool.tile([P, T, H], F32, tag="yg")
                        nc.gpsimd.tensor_tensor(yg[:], harm[:], rcb, ALU.mult)
                        mB = gpool.tile([P, T, H], F32, tag="mB")
                        _meng = nc.gpsimd if (c % 4 < POOL_MAGIC_N) else nc.vector
                        _meng.tensor_scalar(
                            mB[:], yg[:], MAGIC, -MAGIC, ALU.add, ALU.add
                        )
                        r1c = yg
                        nc.vector.tensor_tensor(r1c[:], yg[:], mB[:], ALU.subtract)
                        sv = mB
                        _act(
                            sv[:].rearrange("p t k -> p (t k)"),
                            r1c[:].rearrange("p t k -> p (t k)"),
                            AF.Sin,
                            scale=C2PI,
                            chain=(CHAIN in ("all", "groupall")),
                        )
                        pr = X[:][:, 1]
                        nc.vector.tensor_tensor(pr, sv[:], ha_m, ALU.mult)
                        nc.vector.tensor_reduce(
                            SS_row[:][:, :, s0 : s0 + T],
                            X[:],
                            mybir.AxisListType.X,
                            ALU.add,
                        )

                # row-level normalization and output
                den_r = rpool.tile([P, J], F32, tag="den_r")
                nc.vector.tensor_scalar_add(den_r[:], SS_row[:][:, 0], 1e-5)
                rcp_r = rpool.tile([P, J], F32, tag="rcp_r")
                scr_r = rpool.tile([P, J], F32, tag="scr_r")
                nc.vector.reciprocal_approx_accurate(rcp_r[:], den_r[:], scr_r[:])
                sc_r = den_r
                nc.vector.tensor_tensor(sc_r[:], rcp_r[:], tot_row[:], ALU.mult)
                nc.vector.tensor_tensor(SIG[:], SS_row[:][:, 1], sc_r[:], ALU.mult)

                nc.sync.dma_start(y_d[r].rearrange("(p j) -> p j", p=P), SIG[:])

    nc.compile()
    return nc


def kernel(f0_hz, harmonic_amplitudes, phase):
    from concourse.bass_utils import run_bass_kernel_spmd

    if "nc" not in _CACHE:
        _CACHE["nc"] = _build()
    nc = _CACHE["nc"]

    n_cores = 8
    in_maps = []
    for i in range(n_cores):
        sl = slice(i * ROWS, (i + 1) * ROWS)
        in_maps.append(
            {
                "f0": np.ascontiguousarray(f0_hz[sl], dtype=np.float32),
                "ha": np.ascontiguousarray(
                    harmonic_amplitudes[sl], dtype=np.float32
                ),
                "ph": np.ascontiguousarray(
                    phase[sl], dtype=np.float32
                ).reshape(ROWS, 1),
            }
        )
    res = run_bass_kernel_spmd(nc, in_maps, core_ids=list(range(n_cores)))
    out = np.concatenate([res.results[i]["y"] for i in range(n_cores)], axis=0)
    return out.astype(np.float32)

